# revision 6
# baseline (speedup 1.0000x reference)
"""Trainium2 Bass kernel for nn_AttentionBlock (B=4, C=256, H=W=64, RD=32).

v3: transposed-output attention (v2) + three-way evacuation split.

8 cores = (batch b, query-half h); each core computes out.T for its 2048
queries and the host un-transposes.

Math (per core, b fixed, i in its half, j over all 4096 positions):
  q = Wq x + bq                       [32, Ni]
  k = Wk x + bk                       [32, N]
  vT_aug[j, c'] = (Wv x + bv).T, with column c'=256 equal to 1/gamma
  P[j, i] = exp(k[:,j] . q[:,i])      (unnormalized; fp32 PSUM, bf16 SBUF)
  acc[i, c'] = sum_j P[j, i] * vT_aug[j, c']   (PE, transposed output)
  out.T[i, c] = acc[i, c] * (1 / acc[i, 256]) + x.T[i, c]

v3 changes vs v2 (which ran ACT-bound: ACT 81us busy vs PE 96us in sim):
  - The 64 exp evacuations (PSUM energies -> SBUF bf16 P tiles) are split
    across three engines: ACT computes true exp; DVE and GPSIMD/Pool use a
    Schraudolph bitcast exp (I16 = e*128*log2(e) + 16250.5 truncated,
    reinterpreted as bf16 == 2^(e*log2 e) with ~+-3.5% element error that
    cancels in the softmax ratio; measured end-to-end delta ~5e-4).
  - q and k projections for the query half share one packed stationary
    [wqt|wkt] (one moving stream instead of two); k evacuates into strip 1
    (partitions 32:64) and is replicated to strips 0/2/3 by DMA.
  - Phase-1 projections allocate PSUM from the energy pool's rotating tag
    instead of the accumulator tags, so the next For_i iteration's
    projections overlap the current iteration's attention tail.
  - vt evacuation and the finalize combine are spread over Pool/DVE/ACT
    by static schedules to keep every engine below the PE roofline.
"""

import contextlib
import os
import sys

for _p in ("/opt/trn_rl_repo", "/root/.axon_site/_ro/trn_rl_repo"):
    if os.path.isdir(_p) and _p not in sys.path:
        sys.path.insert(0, _p)

import numpy as np
import ml_dtypes

import concourse.mybir as mybir
import concourse.tile as tile
from concourse import bacc
from concourse.bass_utils import run_bass_kernel_spmd

B, C, H, W = 4, 256, 64, 64
N = H * W            # 4096 positions
RD = C // 8          # 32 reduced dim
NCORES = 8
NI = N // 2          # 2048 queries per core
GW = 512             # i-group width (PSUM bank = 512 fp32)
G = NI // GW         # 4 i-groups
JB = N // 128        # 32 j-blocks
CA = C + 1           # 257: vT columns (256 ch + Z col at index C)

f32 = mybir.dt.float32
i16 = mybir.dt.int16
bf16 = mybir.dt.bfloat16
Exp = mybir.ActivationFunctionType.Exp
Ident = mybir.ActivationFunctionType.Identity
Mult = mybir.AluOpType.mult
AddOp = mybir.AluOpType.add

# Schraudolph bitcast-exp constants (bf16 target):
#   I16 = trunc(e * 128*log2(e) + (127*128 + c + 0.5)); bitcast bf16.
# c = -6 centers the sawtooth error (~[-3.5%, +3.0%]); the +0.5 makes
# the f32->i16 truncation a round-half-up.
SCH_A = 128 * 1.4426950408889634
SCH_B = 128 * 127 - 6.0 + 0.5


def _sched(counts: dict, total: int) -> list:
    """Weighted round-robin engine schedule, e.g. {'A':34,'D':20,'P':10}."""
    assert sum(counts.values()) == total
    acc = {k: 0.0 for k in counts}
    out = []
    for i in range(total):
        k = max(counts, key=lambda e: counts[e] * (i + 1) / total - acc[e])
        acc[k] += 1.0
        out.append(k)
    return out


def build_nc(n_iter: int = 1, pp_bufs: int = 8, prime: int = 4,
             exp_w: int = 2, p_dt=bf16, cad_n: int = 4, cad_ph: int = 1,
             cad_k: int = 2, qk_dt=bf16, outp_bufs: int = 8,
             exp_act: int = 38, exp_dve: int = 26,
             vt_dve: int = 8, vt_act: int = 24,
             fin_stt_dve: int = 8):
    nc = bacc.Bacc()

    xr = nc.dram_tensor("xr", [C, N], qk_dt, kind="ExternalInput")
    xth = nc.dram_tensor("xth", [NI, C], f32, kind="ExternalInput")
    wqkt = nc.dram_tensor("wqkt", [C, 2 * RD], qk_dt, kind="ExternalInput")
    wvt = nc.dram_tensor("wvt", [C, CA], qk_dt, kind="ExternalInput")
    bqk_t = nc.dram_tensor("bqk", [2 * RD, 1], f32, kind="ExternalInput")
    bvz_t = nc.dram_tensor("bvz", [1, CA], f32, kind="ExternalInput")
    one_t = nc.dram_tensor("one_r", [1, 128], f32, kind="ExternalInput")
    out_t = nc.dram_tensor("out", [NI, C], f32, kind="ExternalOutput")

    # GPSIMD/Pool cannot access PSUM, so PSUM-draining work (exp, vt evac)
    # is split across ACT and DVE only; Pool takes SBUF-side finalize adds.
    n_pairs = G * (JB // exp_w)
    exp_sched = _sched({'A': exp_act, 'D': exp_dve}, n_pairs)
    vt_sched = _sched({'D': vt_dve, 'A': vt_act}, JB)
    EW = exp_w * GW

    with tile.TileContext(nc) as tc:
        with tc.tile_pool(name="const", bufs=1) as cp, \
             tc.tile_pool(name="vtp", bufs=1) as vtp, \
             tc.tile_pool(name="qk", bufs=1) as qkp, \
             tc.tile_pool(name="pp", bufs=pp_bufs) as pp, \
             tc.tile_pool(name="fin", bufs=2) as fp, \
             tc.tile_pool(name="outp", bufs=outp_bufs) as op_, \
             tc.tile_pool(name="ps_e", bufs=2, space="PSUM") as ps_e, \
             tc.tile_pool(name="ps_a", bufs=1, space="PSUM") as ps_a:

            # ---- constant loads -------------------------------------------
            xr_sb = [cp.tile([128, N], qk_dt, tag=f"xr{m}", name=f"xr{m}")
                     for m in range(2)]
            for m in range(2):
                nc.sync.dma_start(out=xr_sb[m],
                                  in_=xr[m * 128:(m + 1) * 128, :])
            xth_sb = cp.tile([128, (NI // 128) * C], f32, tag="xth",
                             name="xth_sb")
            for ic in range(NI // 128):
                nc.sync.dma_start(
                    out=xth_sb[:, ic * C:(ic + 1) * C],
                    in_=xth[ic * 128:(ic + 1) * 128, :])
            wqkt_sb = [cp.tile([128, 2 * RD], qk_dt, tag=f"wqkt{m}",
                               name=f"wqkt{m}") for m in range(2)]
            wvt_sb = [cp.tile([128, CA], qk_dt, tag=f"wvt{m}", name=f"wvt{m}")
                      for m in range(2)]
            for m in range(2):
                ms = slice(m * 128, (m + 1) * 128)
                nc.sync.dma_start(out=wqkt_sb[m], in_=wqkt[ms, :])
                nc.sync.dma_start(out=wvt_sb[m], in_=wvt[ms, :])
            bqk_sb = cp.tile([2 * RD, 1], f32, tag="bqk", name="bqk_sb")
            nc.sync.dma_start(out=bqk_sb, in_=bqk_t[:])
            bvz_sb = cp.tile([1, CA], f32, tag="bvz", name="bvz_sb")
            nc.sync.dma_start(out=bvz_sb, in_=bvz_t[:])
            one_sb = cp.tile([1, 128], f32, tag="one", name="one_sb")
            nc.sync.dma_start(out=one_sb, in_=one_t[:])

            # [bv, 1/gamma, 0] broadcast to all 128 partitions (plants the
            # Z column that folds the gamma multiply into the reciprocal)
            pbv = ps_a.tile([128, CA], f32, tag="a0", name="pbv")
            nc.tensor.matmul(pbv, one_sb, bvz_sb, start=True, stop=True)
            bvbc_sb = cp.tile([128, CA], f32, tag="bvbc", name="bvbc_sb")
            nc.vector.tensor_copy(bvbc_sb, pbv)

            # persistent activation tiles; k/q replicated across the four
            # 32-partition strips for packed energy matmuls
            vt = [vtp.tile([128, CA], p_dt, tag=f"vt{jb}", name=f"vt{jb}")
                  for jb in range(JB)]
            for jb in range(JB):
                nc.vector.tensor_copy(vt[jb][:, C:CA], bvbc_sb[:, C:CA])
            q4 = qkp.tile([128, NI], qk_dt, tag="q", name="q4")
            k4 = qkp.tile([128, N], qk_dt, tag="k", name="k4")

            loop_cm = (tc.For_i(0, n_iter, 1) if n_iter > 1
                       else contextlib.nullcontext())
            with loop_cm:
                # ---- phase 1: projections ---------------------------------
                # packed q+k over this core's query half (xr cols 0:NI after
                # the host roll): one moving stream, [wq|wk] stationary.
                # q -> psum rows 0:32 -> q4 strip 0 (ACT, bias via ACT)
                # k -> psum rows 32:64 -> k4 strip 1 (DVE, bias via DVE)
                for g in range(G):
                    gs = slice(g * GW, (g + 1) * GW)
                    pqk = ps_e.tile([64, GW], f32, tag="pe", name="pqk")
                    nc.tensor.matmul(pqk, wqkt_sb[0], xr_sb[0][:, gs],
                                     start=True, stop=False)
                    nc.tensor.matmul(pqk, wqkt_sb[1], xr_sb[1][:, gs],
                                     start=False, stop=True)
                    nc.scalar.activation(q4[0:RD, gs], pqk[0:RD, :], Ident,
                                         bias=bqk_sb[0:RD])
                    nc.vector.tensor_scalar_add(k4[RD:2 * RD, gs],
                                                pqk[RD:2 * RD, :],
                                                bqk_sb[RD:2 * RD])

                # solo k for the other half's positions (xr cols NI:N);
                # stationary = wk columns of the packed tile -> psum rows
                # 32:64 via col tile_position
                for g in range(G, N // GW):
                    gs = slice(g * GW, (g + 1) * GW)
                    pk = ps_e.tile([64, GW], f32, tag="pe", name="pk")
                    nc.tensor.matmul(pk[RD:2 * RD, :],
                                     wqkt_sb[0][:, RD:2 * RD],
                                     xr_sb[0][:, gs], start=True, stop=False,
                                     tile_position=(0, RD))
                    nc.tensor.matmul(pk[RD:2 * RD, :],
                                     wqkt_sb[1][:, RD:2 * RD],
                                     xr_sb[1][:, gs], start=False, stop=True,
                                     tile_position=(0, RD))
                    nc.vector.tensor_scalar_add(k4[RD:2 * RD, gs],
                                                pk[RD:2 * RD, :],
                                                bqk_sb[RD:2 * RD])

                # replicate q (strip 0) to strips 1-3 and k (strip 1) to
                # strips 0,2,3; these DMAs hide under the vT matmuls below
                for t in (1, 2, 3):
                    nc.sync.dma_start(out=q4[RD * t:RD * (t + 1), :],
                                      in_=q4[0:RD, :])
                for t in (0, 2, 3):
                    nc.sync.dma_start(out=k4[RD * t:RD * (t + 1), :],
                                      in_=k4[RD:2 * RD, :])

                # vT_aug j-blocks: x.T @ WvT (+ broadcast [bv,1/gamma,0]);
                # evacuation engine from vt_sched
                for jb in range(JB):
                    js = slice(jb * 128, (jb + 1) * 128)
                    pv = ps_e.tile([128, CA], f32, tag="pe", name="pv")
                    nc.tensor.matmul(pv, xr_sb[0][:, js], wvt_sb[0],
                                     start=True, stop=False)
                    nc.tensor.matmul(pv, xr_sb[1][:, js], wvt_sb[1],
                                     start=False, stop=True)
                    if vt_sched[jb] == 'D':
                        nc.vector.tensor_copy(vt[jb][:, 0:C], pv[:, 0:C])
                    else:
                        nc.scalar.activation(vt[jb][:, 0:C], pv[:, 0:C],
                                             Ident)

                # ---- energy + exp pipeline --------------------------------
                # pair = 2 packed energy matmuls (strips 2m, 2m+1) into one
                # 2-bank PSUM tile, drained by one engine from exp_sched:
                # ACT = true exp; DVE/Pool = Schraudolph bitcast exp
                eq = [(g, jp) for g in range(G) for jp in range(JB // exp_w)]
                p_tiles = {}
                next_e = 0

                def emit_energy_pair():
                    nonlocal next_e
                    if next_e >= len(eq):
                        return
                    g, jp = eq[next_e]
                    eng = exp_sched[next_e]
                    next_e += 1
                    gs = slice(g * GW, (g + 1) * GW)
                    pe2 = ps_e.tile([128, EW], f32, tag="pe", name="pe2")
                    for h in range(exp_w):
                        jc = jp * exp_w + h
                        t = jc % 4
                        js = slice(jc * 128, (jc + 1) * 128)
                        ts_ = slice(32 * t, 32 * (t + 1))
                        nc.tensor.matmul(
                            pe2[:, h * GW:(h + 1) * GW],
                            k4[ts_, js], q4[ts_, gs],
                            start=True, stop=True,
                            tile_position=(32 * t, 0))
                    pt2 = pp.tile([128, EW], p_dt, tag="P", name="pt2")
                    if eng == 'A':
                        nc.scalar.activation(pt2, pe2, Exp)
                    else:
                        nc.vector.tensor_scalar(
                            pt2[:, :].bitcast(i16), pe2, SCH_A, SCH_B,
                            Mult, AddOp)
                    p_tiles[(g, jp)] = pt2

                for _ in range(prime):
                    emit_energy_pair()

                # ---- phase 2: attention (transposed output) ---------------
                for g in range(G):
                    ac = [ps_a.tile([128, CA], f32, tag=f"a{t}",
                                    name=f"ac{t}") for t in range(4)]
                    for jc in range(JB):
                        jp, h = divmod(jc, exp_w)
                        pt2 = p_tiles[(g, jp)]
                        first, last = jc == 0, jc == JB - 1
                        for t in range(4):
                            lo = h * GW + t * 128
                            nc.tensor.matmul(ac[t], pt2[:, lo:lo + 128],
                                             vt[jc], start=first, stop=last)
                        if h == exp_w - 1:
                            p_tiles.pop((g, jp))
                        if jc % cad_n == cad_ph:
                            for _ in range(cad_k):
                                emit_energy_pair()

                    # finalize: per-partition gamma/Z scale + residual;
                    # fin_stt_dve of 16 chunks use single-op DVE STT, the
                    # rest use ACT scale + Pool add
                    for t in range(4):
                        ic = g * 4 + t
                        zr = fp.tile([128, 1], f32, tag="zr", name="zr")
                        nc.vector.reciprocal(zr, ac[t][:, 256:257])
                        ot = op_.tile([128, C], f32, tag="ot", name="ot")
                        if (ic * fin_stt_dve) % 16 >= fin_stt_dve:
                            nc.scalar.activation(ot, ac[t][:, 0:C], Ident,
                                                 scale=zr)
                            nc.gpsimd.tensor_add(
                                ot, ot, xth_sb[:, ic * C:(ic + 1) * C])
                        else:
                            nc.vector.scalar_tensor_tensor(
                                ot, ac[t][:, 0:C], zr,
                                xth_sb[:, ic * C:(ic + 1) * C],
                                Mult, AddOp)
                        nc.sync.dma_start(
                            out=out_t[ic * 128:(ic + 1) * 128, :], in_=ot)
    nc.finalize()
    return nc


_CACHE = {}


def _get_nc(n_iter: int = 1):
    if n_iter not in _CACHE:
        _CACHE[n_iter] = build_nc(n_iter)
    return _CACHE[n_iter]


def make_in_maps(x, Wq, bq, Wk, bk, Wv, bv, gamma):
    x = np.asarray(x, dtype=np.float32)
    Wq = np.asarray(Wq, dtype=np.float32)
    bq = np.asarray(bq, dtype=np.float32)
    Wk = np.asarray(Wk, dtype=np.float32)
    bk = np.asarray(bk, dtype=np.float32)
    Wv = np.asarray(Wv, dtype=np.float32)
    bv = np.asarray(bv, dtype=np.float32)
    gamma = np.asarray(gamma, dtype=np.float32).reshape(())

    bf = ml_dtypes.bfloat16
    wqkt = np.concatenate([Wq.T, Wk.T], axis=1)       # [C, 64]
    wqkt = np.ascontiguousarray(wqkt).astype(bf)
    wvt = np.zeros((C, CA), dtype=np.float32)         # [Wv.T | 0 | 0]
    wvt[:, :C] = Wv.T
    wvt = wvt.astype(bf)
    bvz = np.zeros((1, CA), dtype=np.float32)         # [0, 1/gamma, 0]
    with np.errstate(divide="ignore"):
        bvz[0, C] = np.float32(1.0) / gamma           # inf if gamma==0 ->
    one_r = np.ones((1, 128), dtype=np.float32)       # recip(inf)=0 -> out=x
    bqk = np.concatenate([bq, bk]).reshape(2 * RD, 1)

    in_maps = []
    for c in range(NCORES):
        b, half = divmod(c, 2)
        xb = x[b].reshape(C, N)
        # roll so this core's query half sits at columns 0:NI (the kernel
        # reads queries from xr[:, 0:NI]); k/v use all columns so the roll
        # only permutes j, and the ones-column Z is permutation-invariant
        xbr = np.ascontiguousarray(np.roll(xb, -half * NI, axis=1)).astype(bf)
        xthh = np.ascontiguousarray(
            xb[:, half * NI:(half + 1) * NI].T
            + np.float32(gamma) * bv[None, :])
        in_maps.append({
            "xr": xbr, "xth": xthh,
            "wqkt": wqkt, "wvt": wvt,
            "bqk": bqk, "bvz": bvz, "one_r": one_r,
        })
    return in_maps


def assemble(results):
    out = np.empty((B, C, N), dtype=np.float32)
    for c in range(NCORES):
        b, half = divmod(c, 2)
        out[b][:, half * NI:(half + 1) * NI] = results[c]["out"].T
    return out.reshape(B, C, H, W)


def kernel(x, Wq, bq, Wk, bk, Wv, bv, gamma):
    nc = _get_nc(1)
    in_maps = make_in_maps(x, Wq, bq, Wk, bk, Wv, bv, gamma)
    res = run_bass_kernel_spmd(nc, in_maps, list(range(NCORES)))
    return assemble(res.results)


# revision 38
# speedup vs baseline: 1.0918x; 1.0918x over previous
"""Trainium2 Bass kernel for nn_AttentionBlock (B=4, C=256, H=W=64, RD=32).

v3: transposed-output attention (v2) + three-way evacuation split.

8 cores = (batch b, query-half h); each core computes out.T for its 2048
queries and the host un-transposes.

Math (per core, b fixed, i in its half, j over all 4096 positions):
  q = Wq x + bq                       [32, Ni]
  k = Wk x + bk                       [32, N]
  vT_aug[j, c'] = (Wv x + bv).T, with column c'=256 equal to 1/gamma
  P[j, i] = exp(k[:,j] . q[:,i])      (unnormalized; fp32 PSUM, bf16 SBUF)
  acc[i, c'] = sum_j P[j, i] * vT_aug[j, c']   (PE, transposed output)
  out.T[i, c] = acc[i, c] * (1 / acc[i, 256]) + x.T[i, c]

v7 changes vs v2 (measured on HW: 119.5us -> 114.3us loop-slope; sim
single-shot span 125.4us -> 108.6us):
  - The 64 exp evacuations (PSUM energies -> SBUF bf16 P tiles) split
    38 ACT (true exp) / 26 DVE (Schraudolph bitcast exp: I16 =
    e*128*log2(e) + 16250.5 truncated, reinterpreted as bf16 ==
    2^(e*log2 e); the ~+-3.5% element error largely cancels in the
    softmax ratio; measured end-to-end delta ~2e-4). GPSIMD/Pool cannot
    read PSUM, so it only takes SBUF-side finalize adds.
  - q and k projections for the query half share one packed stationary
    [wqt|wkt] (one moving stream instead of two); k evacuates into strip
    1 (partitions 32:64), q into strip 0, and both are replicated to the
    other strips via the gpsimd SWDGE queue (off the SP queue that
    carries output DMAs), ordered by when phase 2 needs each strip.
  - Constant loads are spread across the SP/ACT/Pool DGE queues so the
    startup chain gating the first projections is ~5us instead of ~17us;
    the residual tensor (only needed at finalize) loads last.
  - q4/k4 are double-buffered per iteration; phase-1 PSUM rotates over
    the four accumulator tags (depth-4 pipeline, v2 style - a shared
    2-buffer ring measurably stalls phase 1).
  - Measured dead ends kept out: fp8/DoubleRow attention (e4m3 has only
    ~10.8 energy-units of Schraudolph dynamic range; e5m2's 2-bit
    mantissa is too coarse), single-strip energy (row-pack concurrency
    is real: +10us without it), 512-wide exp tiles (instruction
    overhead), jc reordering (halves pack concurrency), finalize
    staging via extra PSUM->SBUF copies (queue traffic beats the
    bank-release win). Note For_i places an all-engine barrier in each
    iteration's semaphore-reset block, so loop iterations cannot
    overlap; the loop slope effectively measures the serial body span.
"""

import contextlib
import os
import sys

for _p in ("/opt/trn_rl_repo", "/root/.axon_site/_ro/trn_rl_repo"):
    if os.path.isdir(_p) and _p not in sys.path:
        sys.path.insert(0, _p)

import numpy as np
import ml_dtypes

import concourse.mybir as mybir
import concourse.tile as tile
from concourse import bacc
from concourse.bass_utils import run_bass_kernel_spmd

B, C, H, W = 4, 256, 64, 64
N = H * W            # 4096 positions
RD = C // 8          # 32 reduced dim
NCORES = 8
NI = N // 2          # 2048 queries per core
GW = 512             # i-group width (PSUM bank = 512 fp32)
G = NI // GW         # 4 i-groups
JB = N // 128        # 32 j-blocks
CA = C + 1           # 257: vT columns (256 ch + Z col at index C)

f32 = mybir.dt.float32
i16 = mybir.dt.int16
bf16 = mybir.dt.bfloat16
Exp = mybir.ActivationFunctionType.Exp
Ident = mybir.ActivationFunctionType.Identity
Mult = mybir.AluOpType.mult
AddOp = mybir.AluOpType.add

# Schraudolph bitcast-exp constants (bf16 target):
#   I16 = trunc(e * 128*log2(e) + (127*128 + c + 0.5)); bitcast bf16.
# c = -6 centers the sawtooth error (~[-3.5%, +3.0%]); the +0.5 makes
# the f32->i16 truncation a round-half-up.
SCH_A = 128 * 1.4426950408889634
SCH_B = 128 * 127 - 6.0 + 0.5


def _sched(counts: dict, total: int) -> list:
    """Weighted round-robin engine schedule, e.g. {'A':34,'D':20,'P':10}."""
    assert sum(counts.values()) == total
    acc = {k: 0.0 for k in counts}
    out = []
    for i in range(total):
        k = max(counts, key=lambda e: counts[e] * (i + 1) / total - acc[e])
        acc[k] += 1.0
        out.append(k)
    return out


def build_nc(n_iter: int = 1, pp_bufs: int = 8, prime: int = 4,
             exp_w: int = 2, p_dt=bf16, cad_n: int = 4, cad_ph: int = 1,
             cad_k: int = 2, qk_dt=bf16, outp_bufs: int = 8,
             exp_act: int = 38, exp_dve: int = 26,
             vt_dve: int = 8, vt_act: int = 24,
             fin_stt_dve: int = 8, strips: int = 4,
             reps_gpsimd: bool = True, skip_attn: bool = False,
             skip_exp: bool = False, skip_energy: bool = False,
             skip_p1: bool = False, fin_stage: bool = False,
             jc_reorder: bool = False, par_loads: bool = True):
    nc = bacc.Bacc()

    xr = nc.dram_tensor("xr", [C, N], qk_dt, kind="ExternalInput")
    xth = nc.dram_tensor("xth", [NI, C], f32, kind="ExternalInput")
    wqkt = nc.dram_tensor("wqkt", [C, 2 * RD], qk_dt, kind="ExternalInput")
    wvt = nc.dram_tensor("wvt", [C, CA], qk_dt, kind="ExternalInput")
    bqk_t = nc.dram_tensor("bqk", [2 * RD, 1], f32, kind="ExternalInput")
    bvz_t = nc.dram_tensor("bvz", [1, CA], f32, kind="ExternalInput")
    one_t = nc.dram_tensor("one_r", [1, 128], f32, kind="ExternalInput")
    out_t = nc.dram_tensor("out", [NI, C], f32, kind="ExternalOutput")

    # GPSIMD/Pool cannot access PSUM, so PSUM-draining work (exp, vt evac)
    # is split across ACT and DVE only; Pool takes SBUF-side finalize adds.
    n_pairs = G * (JB // exp_w)
    exp_sched = _sched({'A': exp_act, 'D': exp_dve}, n_pairs)
    vt_sched = _sched({'D': vt_dve, 'A': vt_act}, JB)
    EW = exp_w * GW

    with tile.TileContext(nc) as tc:
        with tc.tile_pool(name="const", bufs=1) as cp, \
             tc.tile_pool(name="vtp", bufs=1) as vtp, \
             tc.tile_pool(name="qk", bufs=2) as qkp, \
             tc.tile_pool(name="pp", bufs=pp_bufs) as pp, \
             tc.tile_pool(name="fin", bufs=2) as fp, \
             tc.tile_pool(name="outp", bufs=outp_bufs) as op_, \
             tc.tile_pool(name="ps_e", bufs=2, space="PSUM") as ps_e, \
             tc.tile_pool(name="ps_a", bufs=1, space="PSUM") as ps_a:

            # ---- constant loads -------------------------------------------
            # spread across the SP/DVE/ACT/Pool DGE queues so the serial
            # startup chain (which gates the first projections) is short;
            # xth is only needed at finalize, so it loads last on SP.
            ld_w = nc.gpsimd if par_loads else nc.sync
            ld_x1 = nc.scalar if par_loads else nc.sync
            xr_sb = [cp.tile([128, N], qk_dt, tag=f"xr{m}", name=f"xr{m}")
                     for m in range(2)]
            nc.sync.dma_start(out=xr_sb[0], in_=xr[0:128, :])
            ld_x1.dma_start(out=xr_sb[1], in_=xr[128:256, :])
            wqkt_sb = [cp.tile([128, 2 * RD], qk_dt, tag=f"wqkt{m}",
                               name=f"wqkt{m}") for m in range(2)]
            wvt_sb = [cp.tile([128, CA], qk_dt, tag=f"wvt{m}", name=f"wvt{m}")
                      for m in range(2)]
            for m in range(2):
                ms = slice(m * 128, (m + 1) * 128)
                ld_w.dma_start(out=wqkt_sb[m], in_=wqkt[ms, :])
                ld_w.dma_start(out=wvt_sb[m], in_=wvt[ms, :])
            bqk_sb = cp.tile([2 * RD, 1], f32, tag="bqk", name="bqk_sb")
            ld_w.dma_start(out=bqk_sb, in_=bqk_t[:])
            bvz_sb = cp.tile([1, CA], f32, tag="bvz", name="bvz_sb")
            ld_w.dma_start(out=bvz_sb, in_=bvz_t[:])
            one_sb = cp.tile([1, 128], f32, tag="one", name="one_sb")
            ld_w.dma_start(out=one_sb, in_=one_t[:])
            xth_sb = cp.tile([128, (NI // 128) * C], f32, tag="xth",
                             name="xth_sb")
            for ic in range(NI // 128):
                nc.sync.dma_start(
                    out=xth_sb[:, ic * C:(ic + 1) * C],
                    in_=xth[ic * 128:(ic + 1) * 128, :])

            # [bv, 1/gamma, 0] broadcast to all 128 partitions (plants the
            # Z column that folds the gamma multiply into the reciprocal)
            pbv = ps_a.tile([128, CA], f32, tag="a0", name="pbv")
            nc.tensor.matmul(pbv, one_sb, bvz_sb, start=True, stop=True)
            bvbc_sb = cp.tile([128, CA], f32, tag="bvbc", name="bvbc_sb")
            nc.vector.tensor_copy(bvbc_sb, pbv)

            # persistent activation tiles; k/q replicated across the four
            # 32-partition strips for packed energy matmuls
            vt = [vtp.tile([128, CA], p_dt, tag=f"vt{jb}", name=f"vt{jb}")
                  for jb in range(JB)]
            for jb in range(JB):
                nc.vector.tensor_copy(vt[jb][:, C:CA], bvbc_sb[:, C:CA])
            loop_cm = (tc.For_i(0, n_iter, 1) if n_iter > 1
                       else contextlib.nullcontext())
            with loop_cm:
                # double-buffered per-iteration activations: iteration n+1's
                # projections don't wait on iteration n's last energy matmul
                qrows = 128 if strips == 4 else 64
                q4 = qkp.tile([qrows, NI], qk_dt, tag="q", name="q4")
                k4 = qkp.tile([qrows, N], qk_dt, tag="k", name="k4")
                rep_eng = nc.gpsimd if reps_gpsimd else nc.sync
                # ---- phase 1: projections ---------------------------------
                # packed q+k over this core's query half (xr cols 0:NI after
                # the host roll): one moving stream, [wq|wk] stationary.
                # q -> psum rows 0:32 -> q4 strip 0 (ACT, bias via ACT)
                # k -> psum rows 32:64 -> k4 strip 1 (DVE, bias via DVE)
                # phase-1 PSUM rotates over the four accumulator tags (depth
                # 4); the staged finalize below frees those banks right after
                # each group's last attention matmul
                for g in range(0 if skip_p1 else G):
                    gs = slice(g * GW, (g + 1) * GW)
                    pqk = ps_a.tile([64, GW], f32, tag=f"a{g % 4}",
                                    name="pqk")
                    nc.tensor.matmul(pqk, wqkt_sb[0], xr_sb[0][:, gs],
                                     start=True, stop=False)
                    nc.tensor.matmul(pqk, wqkt_sb[1], xr_sb[1][:, gs],
                                     start=False, stop=True)
                    nc.scalar.activation(q4[0:RD, gs], pqk[0:RD, :], Ident,
                                         bias=bqk_sb[0:RD])
                    nc.vector.tensor_scalar_add(k4[RD:2 * RD, gs],
                                                pqk[RD:2 * RD, :],
                                                bqk_sb[RD:2 * RD])

                # solo k for the other half's positions (xr cols NI:N);
                # stationary = wk columns of the packed tile -> psum rows
                # 32:64 via col tile_position
                for g in range(G if not skip_p1 else N // GW, N // GW):
                    gs = slice(g * GW, (g + 1) * GW)
                    pk = ps_a.tile([64, GW], f32, tag=f"a{g % 4}", name="pk")
                    nc.tensor.matmul(pk[RD:2 * RD, :],
                                     wqkt_sb[0][:, RD:2 * RD],
                                     xr_sb[0][:, gs], start=True, stop=False,
                                     tile_position=(0, RD))
                    nc.tensor.matmul(pk[RD:2 * RD, :],
                                     wqkt_sb[1][:, RD:2 * RD],
                                     xr_sb[1][:, gs], start=False, stop=True,
                                     tile_position=(0, RD))
                    nc.vector.tensor_scalar_add(k4[RD:2 * RD, gs],
                                                pk[RD:2 * RD, :],
                                                bqk_sb[RD:2 * RD])

                # replicate q (strip 0) to the other strips and k (strip 1)
                # likewise; issued via the gpsimd SWDGE queue so they don't
                # queue behind the previous iteration's output DMAs on SP.
                # Ordered by when phase 2 needs each strip (q1, k0 first);
                # k replications split in column halves so the first energy
                # pairs never wait on the tail of the replication queue.
                # strips=1: no tile_position packing; q joins k on
                # partitions 32:64 with a single replication.
                if strips == 4:
                    h1, h2 = slice(0, NI), slice(NI, N)
                    rep_eng.dma_start(out=q4[RD:2 * RD, :], in_=q4[0:RD, :])
                    rep_eng.dma_start(out=k4[0:RD, h1], in_=k4[RD:2 * RD, h1])
                    for t in (2, 3):
                        rep_eng.dma_start(out=q4[RD * t:RD * (t + 1), :],
                                          in_=q4[0:RD, :])
                        rep_eng.dma_start(out=k4[RD * t:RD * (t + 1), h1],
                                          in_=k4[RD:2 * RD, h1])
                    for t in (0, 2, 3):
                        rep_eng.dma_start(out=k4[RD * t:RD * (t + 1), h2],
                                          in_=k4[RD:2 * RD, h2])
                else:
                    rep_eng.dma_start(out=q4[RD:2 * RD, :], in_=q4[0:RD, :])

                # vT_aug j-blocks: x.T @ WvT (+ broadcast [bv,1/gamma,0]);
                # evacuation engine from vt_sched
                for jb in range(0 if skip_p1 else JB):
                    js = slice(jb * 128, (jb + 1) * 128)
                    pv = ps_a.tile([128, CA], f32, tag=f"a{jb % 4}",
                                   name="pv")
                    nc.tensor.matmul(pv, xr_sb[0][:, js], wvt_sb[0],
                                     start=True, stop=False)
                    nc.tensor.matmul(pv, xr_sb[1][:, js], wvt_sb[1],
                                     start=False, stop=True)
                    if vt_sched[jb] == 'D':
                        nc.vector.tensor_copy(vt[jb][:, 0:C], pv[:, 0:C])
                    else:
                        nc.scalar.activation(vt[jb][:, 0:C], pv[:, 0:C],
                                             Ident)

                # ---- energy + exp pipeline --------------------------------
                # pair = 2 packed energy matmuls (strips 2m, 2m+1) into one
                # 2-bank PSUM tile, drained by one engine from exp_sched:
                # ACT = true exp; DVE/Pool = Schraudolph bitcast exp
                # jc_reorder: within each group, process even-strip pairs
                # (row groups 0/1) before odd-strip pairs (2/3) so the first
                # energy matmuls never wait on strip-2/3 replication DMAs
                npg = JB // exp_w
                if jc_reorder:
                    jp_order = ([jp for jp in range(npg) if jp % 2 == 0]
                                + [jp for jp in range(npg) if jp % 2 == 1])
                else:
                    jp_order = list(range(npg))
                eq = [(g, jp) for g in range(G) for jp in jp_order]
                p_tiles = {}
                next_e = 0

                def emit_energy_pair():
                    nonlocal next_e
                    if next_e >= len(eq):
                        return
                    g, jp = eq[next_e]
                    eng = exp_sched[next_e]
                    next_e += 1
                    gs = slice(g * GW, (g + 1) * GW)
                    pe2 = ps_e.tile([128, EW], f32, tag="pe", name="pe2")
                    ew = 8 if skip_energy else GW
                    for h in range(exp_w):
                        jc = jp * exp_w + h
                        t = jc % 4 if strips == 4 else 1
                        js = slice(jc * 128, (jc + 1) * 128)
                        ts_ = slice(32 * t, 32 * (t + 1))
                        nc.tensor.matmul(
                            pe2[:, h * GW:h * GW + ew],
                            k4[ts_, js], q4[ts_, gs][:, 0:ew],
                            start=True, stop=True,
                            tile_position=(32 * t, 0))
                    pt2 = pp.tile([128, EW], p_dt, tag="P", name="pt2")
                    if skip_exp:
                        nc.scalar.activation(pt2[:, 0:8], pe2[:, 0:8], Exp)
                    elif eng == 'A':
                        nc.scalar.activation(pt2, pe2, Exp)
                    else:
                        nc.vector.tensor_scalar(
                            pt2[:, :].bitcast(i16), pe2, SCH_A, SCH_B,
                            Mult, AddOp)
                    p_tiles[(g, jp)] = pt2

                for _ in range(prime):
                    emit_energy_pair()

                # ---- phase 2: attention (transposed output) ---------------
                for g in range(G):
                    ac = [ps_a.tile([128, CA], f32, tag=f"a{t}",
                                    name=f"ac{t}") for t in range(4)]
                    jc_seq = [jp * exp_w + h for jp in jp_order
                              for h in range(exp_w)]
                    for njc, jc in enumerate(jc_seq):
                        jp, h = divmod(jc, exp_w)
                        pt2 = p_tiles[(g, jp)]
                        first, last = jc == 0, jc == JB - 1
                        for t in range(4):
                            lo = h * GW + t * 128
                            if skip_attn:
                                if first:
                                    nc.tensor.matmul(
                                        ac[t][:, 0:8], pt2[:, lo:lo + 128],
                                        vt[jc][:, 0:8], start=True, stop=True)
                            else:
                                nc.tensor.matmul(ac[t], pt2[:, lo:lo + 128],
                                                 vt[jc], start=first,
                                                 stop=last)
                        if h == exp_w - 1:
                            p_tiles.pop((g, jp))
                        if jc % cad_n == cad_ph:
                            for _ in range(cad_k):
                                emit_energy_pair()

                    # finalize: per-partition gamma/Z scale + residual.
                    # With fin_stage, each accumulator is first copied to an
                    # SBUF staging tile in a single op (alternating ACT/DVE)
                    # so its PSUM bank frees immediately for the next group's
                    # matmuls / next iteration's projections; the recip +
                    # combine chain then runs off the critical path.
                    stages = []
                    for t in range(4):
                        if not fin_stage:
                            stages.append(ac[t])
                            continue
                        st = fp.tile([128, CA], f32, tag=f"st{t % 2}",
                                     name="st")
                        if t % 2 == 0:
                            nc.scalar.activation(st, ac[t], Ident)
                        else:
                            nc.vector.tensor_copy(st, ac[t])
                        stages.append(st)
                    for t in range(4):
                        ic = g * 4 + t
                        src = stages[t]
                        zr = fp.tile([128, 1], f32, tag="zr", name="zr")
                        nc.vector.reciprocal(zr, src[:, 256:257])
                        ot = op_.tile([128, C], f32, tag="ot", name="ot")
                        if (ic * fin_stt_dve) % 16 >= fin_stt_dve:
                            nc.scalar.activation(ot, src[:, 0:C], Ident,
                                                 scale=zr)
                            nc.gpsimd.tensor_add(
                                ot, ot, xth_sb[:, ic * C:(ic + 1) * C])
                        else:
                            nc.vector.scalar_tensor_tensor(
                                ot, src[:, 0:C], zr,
                                xth_sb[:, ic * C:(ic + 1) * C],
                                Mult, AddOp)
                        nc.sync.dma_start(
                            out=out_t[ic * 128:(ic + 1) * 128, :], in_=ot)
    nc.finalize()
    return nc


_CACHE = {}


def _get_nc(n_iter: int = 1):
    if n_iter not in _CACHE:
        _CACHE[n_iter] = build_nc(n_iter)
    return _CACHE[n_iter]


def make_in_maps(x, Wq, bq, Wk, bk, Wv, bv, gamma):
    x = np.asarray(x, dtype=np.float32)
    Wq = np.asarray(Wq, dtype=np.float32)
    bq = np.asarray(bq, dtype=np.float32)
    Wk = np.asarray(Wk, dtype=np.float32)
    bk = np.asarray(bk, dtype=np.float32)
    Wv = np.asarray(Wv, dtype=np.float32)
    bv = np.asarray(bv, dtype=np.float32)
    gamma = np.asarray(gamma, dtype=np.float32).reshape(())

    bf = ml_dtypes.bfloat16
    wqkt = np.concatenate([Wq.T, Wk.T], axis=1)       # [C, 64]
    wqkt = np.ascontiguousarray(wqkt).astype(bf)
    wvt = np.zeros((C, CA), dtype=np.float32)         # [Wv.T | 0 | 0]
    wvt[:, :C] = Wv.T
    wvt = wvt.astype(bf)
    bvz = np.zeros((1, CA), dtype=np.float32)         # [0, 1/gamma, 0]
    with np.errstate(divide="ignore"):
        bvz[0, C] = np.float32(1.0) / gamma           # inf if gamma==0 ->
    one_r = np.ones((1, 128), dtype=np.float32)       # recip(inf)=0 -> out=x
    bqk = np.concatenate([bq, bk]).reshape(2 * RD, 1)

    in_maps = []
    for c in range(NCORES):
        b, half = divmod(c, 2)
        xb = x[b].reshape(C, N)
        # roll so this core's query half sits at columns 0:NI (the kernel
        # reads queries from xr[:, 0:NI]); k/v use all columns so the roll
        # only permutes j, and the ones-column Z is permutation-invariant
        xbr = np.ascontiguousarray(np.roll(xb, -half * NI, axis=1)).astype(bf)
        xthh = np.ascontiguousarray(
            xb[:, half * NI:(half + 1) * NI].T
            + np.float32(gamma) * bv[None, :])
        in_maps.append({
            "xr": xbr, "xth": xthh,
            "wqkt": wqkt, "wvt": wvt,
            "bqk": bqk, "bvz": bvz, "one_r": one_r,
        })
    return in_maps


def assemble(results):
    out = np.empty((B, C, N), dtype=np.float32)
    for c in range(NCORES):
        b, half = divmod(c, 2)
        out[b][:, half * NI:(half + 1) * NI] = results[c]["out"].T
    return out.reshape(B, C, H, W)


def kernel(x, Wq, bq, Wk, bk, Wv, bv, gamma):
    nc = _get_nc(1)
    in_maps = make_in_maps(x, Wq, bq, Wk, bk, Wv, bv, gamma)
    res = run_bass_kernel_spmd(nc, in_maps, list(range(NCORES)))
    return assemble(res.results)
